# revision 1
# baseline (speedup 1.0000x reference)
"""Trainium2 Bass kernel for nn_Attention (non-local-block style attention).

Reference computation (per batch b, z flattened to [Ci, N], N = T*H*W = 4096):
    theta = w_theta @ z + b_theta        [Co, N]
    phi   = w_phi   @ z + b_phi          [Co, N]
    psi   = w_psi   @ z + b_psi          [Co, N]
    g[n,m]   = sum_c phi[c,n] psi[c,m]
    G        = relu(g / N)
    tmp[c,n] = sum_m G[n,m] theta[c,m]
    out      = w_v @ tmp + b_v + z       [Ci, N]

Sharding: 8 cores = 2 batches x 4 token-blocks of N/4=1024. Each core gets the
full z[b] (needed for psi/theta over all m) plus its own token block, computes
out[b][:, blk]. Fully data-parallel SPMD, no collectives.

Per-core dataflow. The attention matmuls run in bf16 (full PE rate)
accumulating into fp32 PSUM; the 1/N normalization is folded into w_psi
host-side; the residual path stays fp32.

  psi_dup [128, 4096] bf16: projection with host-duplicated weight columns so
          rows 0-63 == rows 64-127 == psi/N (gives both array row-groups their
          operands for free)
  phi_dup [128, 1024] bf16: same for phi on the core's token block
  thT     [128, 32*64] bf16: theta^T tiles (m on partitions), theta bias added
          via ones-row matmul prefill of each PSUM bank
  per m-tile (32):
    gT [128, 1024] = psi^T phi: n-chunk 0 on array rows 0-63, chunk 1 on rows
       64-127 (tile_position row-group pair, concurrent -> full array; K=64
       matmuls alone never trigger the PE clock-gate warm-up)
    G = relu(gT) PSUM->SBUF bf16, split ScalarE [0:560] / VectorE [560:1024]
    tmp [128, 512] += thT^T G: col-group pair (0,0)/(0,64), both n-chunks
       accumulate into one PSUM bank (rows 0:64 = chunk0, 64:128 = chunk1)
  vg = w_v^T tmp (f32r, row-group pair); out = vg + b_v + z_blk (fp32); DMA.

Schedule: a ~3.5us burst of same-weight matmuls ignites the PE clock gate
(1.2 -> 2.4 GHz) while inputs DMA in; the DMA-paced projection tiles are then
interleaved between attention-loop iterations (software-pipelined 3 tiles
ahead) so no engine ever idles long enough to re-throttle the clock.
"""

import ml_dtypes
import numpy as np

import concourse.bacc as bacc
import concourse.mybir as mybir
import concourse.tile as tile
from concourse.bass_utils import run_bass_kernel_spmd

F32 = mybir.dt.float32
F32R = mybir.dt.float32r
BF16 = mybir.dt.bfloat16
AF = mybir.ActivationFunctionType
ALU = mybir.AluOpType
BF16NP = ml_dtypes.bfloat16

B, CI, CO = 2, 128, 64
T, H, W = 4, 32, 32
N = T * H * W            # 4096 tokens
NCORES = 8
BLK = N // (NCORES // B)  # 1024 tokens per core
CH = 512                 # psum-bank chunk
MT = N // 128            # 32 m-tiles

_CACHE = {}


def _build():
    nc = bacc.Bacc("TRN2", target_bir_lowering=False, debug=False)

    zb16 = nc.dram_tensor("zb16", [CI, N], BF16, kind="ExternalInput")
    zblk16 = nc.dram_tensor("zblk16", [CI, BLK], BF16, kind="ExternalInput")
    zblk = nc.dram_tensor("zblk", [CI, BLK], F32, kind="ExternalInput")
    wpack = nc.dram_tensor("wpack", [CI, 320], BF16, kind="ExternalInput")
    smallpack = nc.dram_tensor("smallpack", [1, 640], BF16, kind="ExternalInput")
    biaspack = nc.dram_tensor("biaspack", [CI, 4], F32, kind="ExternalInput")
    wvT2 = nc.dram_tensor("wvT2", [128, CI], F32R, kind="ExternalInput")
    out = nc.dram_tensor("out", [CI, BLK], F32, kind="ExternalOutput")

    with tile.TileContext(nc) as tc:
        with (
            tc.tile_pool(name="const", bufs=1) as cpool,
            tc.tile_pool(name="zp", bufs=1) as zp,
            tc.tile_pool(name="proj", bufs=1) as pp,
            tc.tile_pool(name="gs", bufs=8) as gp,
            tc.tile_pool(name="tail", bufs=2) as tailp,
            tc.tile_pool(name="pst", bufs=1, space="PSUM") as pst,
        ):
            # ---- loads: 3 packed DMAs for the small tensors, then the z
            # tensors in consumption order, all on the sync queue. Few DMA
            # instructions keep the sync issue rate and the instruction-
            # fetch DMAs off the critical path. ----
            wpack_sb = cpool.tile([CI, 320], BF16)
            nc.sync.dma_start(wpack_sb[:], wpack[:])
            zblk16_sb = zp.tile([CI, BLK], BF16)
            nc.sync.dma_start(zblk16_sb[:], zblk16[:])
            smallpack_sb = cpool.tile([1, 640], BF16)
            nc.sync.dma_start(smallpack_sb[:], smallpack[:])
            biaspack_sb = cpool.tile([CI, 4], F32)
            nc.sync.dma_start(biaspack_sb[:], biaspack[:])
            zb16_sb = zp.tile([CI, N], BF16)
            for j in range(2):
                nc.sync.dma_start(
                    zb16_sb[:, j * 2048:(j + 1) * 2048],
                    zb16[:, j * 2048:(j + 1) * 2048],
                )
            wpsiT2_sb = wpack_sb[:, 0:128]
            wphiT2_sb = wpack_sb[:, 128:256]
            wthetaT_sb = wpack_sb[:, 256:320]
            btheta8_sb = smallpack_sb[:, 0:CH]
            ones_sb = smallpack_sb[:, CH:CH + CI]
            bpsi_sb = biaspack_sb[:, 0:1]
            bphi_sb = biaspack_sb[:, 1:2]
            bv_sb = biaspack_sb[:, 2:3]
            zero_sb = biaspack_sb[:, 3:4]
            # tail-only inputs ride the (slow-starting) gpsimd queue
            wvT2_sb = cpool.tile([128, CI], F32R)
            nc.gpsimd.dma_start(wvT2_sb[:], wvT2[:])
            zblk_sb = zp.tile([CI, BLK], F32)
            nc.gpsimd.dma_start(zblk_sb[:], zblk[:])

            # tmp accumulator: one PSUM bank, col-packed
            # (rows 0:64 = tmp[:, 0:512], rows 64:128 = tmp[:, 512:1024])
            tmp_ps = pst.tile([128, CH], F32)

            psi_sb = pp.tile([128, N], BF16)
            phi_sb = pp.tile([128, BLK], BF16)
            thT_sb = pp.tile([128, MT * CO], BF16)

            # ---- HAM ignition: dense same-weight matmuls during the DMA
            # phase push the PE activity monitor over its busy threshold so
            # the array clock ramps 1.2 -> 2.4 GHz before the real work.
            with tc.tile_pool(name="warm", bufs=1, space="PSUM") as wpool:
                wps = wpool.tile([128, CH], F32)
                for k in range(36):
                    nc.tensor.matmul(
                        wps[:, 0:128], wpsiT2_sb, wpsiT2_sb,
                        skip_group_check=True,
                    )

            # ---- merged projection + attention schedule.
            # psi/theta^T projection tiles are interleaved between main-loop
            # iterations so the DMA-paced projection overlaps the relu-bound
            # attention loop. One PSUM bank (psj, bufs=1) serves the
            # projection tiles; spacing between emissions keeps its
            # recycle latency off the PE critical path.
            if True:
                def emit_psi(j, pool):
                    ps = pool.tile([128, CH], F32, tag="m", name=f"psi{j}")
                    nc.tensor.matmul(
                        ps[:], wpsiT2_sb, zb16_sb[:, j * CH:(j + 1) * CH]
                    )
                    dst = psi_sb[:, j * CH:(j + 1) * CH]
                    if j % 2 == 0:
                        nc.scalar.activation(
                            dst, ps[:], AF.Identity, bias=bpsi_sb
                        )
                    else:
                        nc.vector.tensor_scalar_add(dst, ps[:], bpsi_sb)

                def emit_phi(j, pool):
                    ps = pool.tile([128, CH], F32, tag="m", name=f"phi{j}")
                    nc.tensor.matmul(
                        ps[:], wphiT2_sb, zblk16_sb[:, j * CH:(j + 1) * CH]
                    )
                    dst = phi_sb[:, j * CH:(j + 1) * CH]
                    if j % 2 == 0:
                        nc.scalar.activation(
                            dst, ps[:], AF.Identity, bias=bphi_sb
                        )
                    else:
                        nc.vector.tensor_scalar_add(dst, ps[:], bphi_sb)

                def emit_thT(grp, pool):
                    ps = pool.tile([128, CH], F32, tag="m", name=f"th{grp}")
                    nc.tensor.matmul(
                        ps[:], ones_sb, btheta8_sb,
                        start=True, stop=False, skip_group_check=True,
                    )
                    for j in range(8):
                        mi = grp * 8 + j
                        nc.tensor.matmul(
                            ps[:, j * CO:(j + 1) * CO],
                            zb16_sb[:, mi * 128:(mi + 1) * 128],
                            wthetaT_sb,
                            start=False, stop=(j == 7), skip_group_check=True,
                        )
                    dst = thT_sb[:, grp * CH:(grp + 1) * CH]
                    if grp % 2 == 0:
                        nc.vector.tensor_copy(dst, ps[:])
                    else:
                        nc.scalar.activation(dst, ps[:], AF.Copy)

                # pre-loop projections in their own 2-bank pool (pipelined
                # PSUM recycle), closed before the loop pools open
                with tc.tile_pool(name="psj2", bufs=2, space="PSUM") as psj2:
                    emit_phi(0, psj2)
                    emit_phi(1, psj2)
                    emit_psi(0, psj2)
                    emit_psi(1, psj2)
                    emit_thT(0, psj2)

            with (
                tc.tile_pool(name="psj", bufs=1, space="PSUM") as psj,
                tc.tile_pool(name="psg", bufs=3, space="PSUM") as psg,
            ):
                gsb = {}

                def emit_g(mt):
                    gps = psg.tile([128, 2 * CH], F32, tag="g", name=f"g{mt}")
                    msl = slice(mt * 128, (mt + 1) * 128)
                    nc.tensor.matmul(
                        gps[:, 0:CH],
                        psi_sb[0:CO, msl],
                        phi_sb[0:CO, 0:CH],
                        tile_position=(0, 0),
                    )
                    nc.tensor.matmul(
                        gps[:, CH:2 * CH],
                        psi_sb[CO:128, msl],
                        phi_sb[CO:128, CH:2 * CH],
                        tile_position=(64, 0),
                    )
                    s = gp.tile([128, 2 * CH], BF16, tag="gs", name=f"s{mt}")
                    # split at 560 (not 512) so ScalarE (1.2 GHz) and
                    # VectorE (0.96 GHz) finish together
                    nc.scalar.activation(s[:, 0:560], gps[:, 0:560], AF.Relu)
                    nc.vector.tensor_scalar_max(
                        s[:, 560:2 * CH], gps[:, 560:2 * CH], 0.0
                    )
                    gsb[mt] = s

                def emit_tmp(mt):
                    s = gsb.pop(mt)
                    lhs = thT_sb[:, mt * CO:(mt + 1) * CO]
                    nc.tensor.matmul(
                        tmp_ps[0:CO, :], lhs, s[:, 0:CH],
                        start=(mt == 0), stop=(mt == MT - 1),
                        tile_position=(0, 0), skip_group_check=True,
                    )
                    nc.tensor.matmul(
                        tmp_ps[CO:128, :], lhs, s[:, CH:2 * CH],
                        start=(mt == 0), stop=(mt == MT - 1),
                        tile_position=(0, 64), skip_group_check=True,
                    )

                # pre-loop projections get their own 2-bank pool so their
                # PSUM recycles pipeline (the shared psj bank serializes)
                emit_g(0)
                emit_g(1)
                emit_g(2)
                emit_g(3)
                proj_sched = {
                    0: lambda: emit_psi(2, psj),
                    2: lambda: emit_psi(3, psj),
                    4: lambda: emit_thT(1, psj),
                    6: lambda: emit_psi(4, psj),
                    9: lambda: emit_psi(5, psj),
                    11: lambda: emit_thT(2, psj),
                    14: lambda: emit_psi(6, psj),
                    17: lambda: emit_psi(7, psj),
                    20: lambda: emit_thT(3, psj),
                }
                for mt in range(MT):
                    emit_tmp(mt)
                    if mt in proj_sched:
                        proj_sched[mt]()
                    if mt + 4 < MT:
                        emit_g(mt + 4)

            # ---- tail: tmp -> SBUF, vg = w_v^T tmp (row-packed), out ----
            with tc.tile_pool(name="psv", bufs=2, space="PSUM") as psv:
                tmp_sb = tailp.tile([128, CH], F32R, tag="tmp")
                nc.scalar.activation(tmp_sb[:], tmp_ps[:], AF.Copy)
                vgA = psv.tile([CI, CH], F32, tag="v", name="vgA")
                vgB = psv.tile([CI, CH], F32, tag="v", name="vgB")
                nc.tensor.matmul(
                    vgA[:], wvT2_sb[0:CO, :], tmp_sb[0:CO, :], tile_position=(0, 0)
                )
                nc.tensor.matmul(
                    vgB[:], wvT2_sb[CO:128, :], tmp_sb[CO:128, :],
                    tile_position=(64, 0),
                )
                for h, vg_ps in ((0, vgA), (1, vgB)):
                    out_sb = tailp.tile([CI, CH], F32, tag="os", name=f"os{h}")
                    nc.vector.scalar_tensor_tensor(
                        out_sb[:],
                        vg_ps[:],
                        bv_sb,
                        zblk_sb[:, h * CH:(h + 1) * CH],
                        ALU.add,
                        ALU.add,
                    )
                    nc.sync.dma_start(out[:, h * CH:(h + 1) * CH], out_sb[:])

    nc.compile()
    return nc


def _get_nc():
    if "nc" not in _CACHE:
        _CACHE["nc"] = _build()
    return _CACHE["nc"]


def build_in_maps(z, w_theta, b_theta, w_phi, b_phi, w_psi, b_psi, w_v, b_v):
    z = np.asarray(z, dtype=np.float32)
    z2 = np.ascontiguousarray(z.reshape(B, CI, N))
    z216 = z2.astype(BF16NP)

    sc = np.float32(1.0 / N)
    wpsiT = np.asarray(w_psi, np.float32).T * sc
    wphiT = np.asarray(w_phi, np.float32).T
    wthetaT = np.asarray(w_theta, np.float32).T
    wpack = np.ascontiguousarray(
        np.concatenate(
            [wpsiT, wpsiT, wphiT, wphiT, wthetaT], axis=1
        ).astype(BF16NP)
    )
    smallpack = np.zeros((1, 640), dtype=BF16NP)
    smallpack[0, 0:CH] = np.tile(np.asarray(b_theta, np.float32), 8).astype(BF16NP)
    smallpack[0, CH:CH + CI] = np.ones(CI, dtype=BF16NP)
    biaspack = np.stack(
        [
            np.concatenate([b_psi, b_psi]).astype(np.float32) * sc,
            np.concatenate([b_phi, b_phi]).astype(np.float32),
            np.asarray(b_v, np.float32),
            np.zeros(CI, np.float32),
        ],
        axis=1,
    ).astype(np.float32)
    wvT1 = np.asarray(w_v, np.float32).T
    wvT2 = np.ascontiguousarray(np.concatenate([wvT1, wvT1], axis=0))

    in_maps = []
    for core in range(NCORES):
        b, nb = divmod(core, NCORES // B)
        sl = slice(nb * BLK, (nb + 1) * BLK)
        in_maps.append(
            {
                "zb16": z216[b],
                "zblk16": np.ascontiguousarray(z216[b][:, sl]),
                "zblk": np.ascontiguousarray(z2[b][:, sl]),
                "wpack": wpack,
                "smallpack": smallpack,
                "biaspack": biaspack,
                "wvT2": wvT2,
            }
        )
    return in_maps


def kernel(z, w_theta, b_theta, w_phi, b_phi, w_psi, b_psi, w_v, b_v):
    in_maps = build_in_maps(
        z, w_theta, b_theta, w_phi, b_phi, w_psi, b_psi, w_v, b_v
    )
    nc = _get_nc()
    res = run_bass_kernel_spmd(nc, in_maps, core_ids=list(range(NCORES)))

    out_full = np.empty((B, CI, N), dtype=np.float32)
    for core in range(NCORES):
        b, nb = divmod(core, NCORES // B)
        out_full[b][:, nb * BLK:(nb + 1) * BLK] = res.results[core]["out"]
    return out_full.reshape(B, CI, T, H, W)

